# revision 91
# baseline (speedup 1.0000x reference)
"""Trainium2 Bass kernel for nn_LinearTransformerBlock_27041114096177.

Linear-attention transformer block: q/k/v projections + RoPE + ReLU feature
map, per-head (value,key) outer-product state, eps-normalized readout, and an
output projection.

Sharding (8 NeuronCores): data-parallel over batch (2 groups of 4 cores) x
tensor-parallel over heads (4 of 16 heads per core). Weights are pre-sharded
and pre-transposed on host. Each core computes a partial out.T = Wo_shard @
hs_norm for its heads over the full sequence; the host sums the 4 partials per
batch (the "all-reduce" of the out-projection) and transposes back.

Per-core dataflow (fp32/f32r, PE-centric):
  phase 1, per 128-row s-tile:
    xT tile [128c,16,128s] (f32r, stationary) x WqkvT slices -> q/k/v PSUM
    RoPE via strided even/odd DVE muls + add; ACT relu
    vk[h] PSUM [d,129] += kr_h.T-contraction vs [v_h | ones]  (K = s)
    PE-transpose (f32r) of qr head blocks -> qrT stash in SBUF (ACT copy)
  phase 2 (qrT read straight from SBUF): the per-s normalization scales
  columns of hs, so it commutes past Wo — fold vk into the out-projection
  weights once (wT[d,c] = sum_n vk[n,d] Wo[c,n], built per wo quarter as it
  streams) and normalize qT instead:
    den chains: tmp = qT*vk_ones (DVE) -> partition_all_reduce (gpsimd) ->
    recip (DVE) -> qhat = qT*recip (DVE), 3 chunks ahead of the out-proj
    outT[j-tile, chunk] PSUM += wT tiles @ qhat[h], h=0..3 -> DMA out

f32r notes (verified on hw): DRAM params may be declared f32r directly over
host fp32 bytes; ACT/DVE writes to f32r tiles apply the required f32r
rounding, but f32->f32r bitcasts do not (BIR verifier rejects unrounded
producers); f32r->f32 bitcast on matmul operands is fine; f32r matmuls with
lhsT free dim 1/2/65 do not compile, and f32r only reaches 1 cycle/row when
the output free dim is >= 256 (hence the vk rhs zero-padded to 256 cols).

Biases bq/bk/bv are all-zero by the problem spec (fill: zeros) and are not
applied on-device; bo is added on host.
"""
import os
import sys
import time

sys.path.insert(0, '/opt/trn_rl_repo')

import numpy as np
from contextlib import ExitStack

import concourse.bass as bass
import concourse.bacc as bacc
from concourse import bass_isa
import concourse.tile as tile
from concourse import mybir
from concourse.masks import make_identity

B, S, C = 2, 4096, 2048
H, D = 16, 128
HPC = 4              # heads per core
M = HPC * D          # 512: per-core projection width
NCORES = 8
TPG = 4              # cores per batch group
ST = S // 128        # 32 s-tiles of 128
CT = C // 128        # 16 contraction tiles
NCH = S // 512       # 8 s-chunks of 512
EPS = 1e-15

F32 = mybir.dt.float32
F32R = mybir.dt.float32r
BF16 = mybir.dt.bfloat16

_CACHE = {}
LAST_STATS = {}


def _rope_relu(nc, pools, ps, cos_t, sin_t, out_sb):
    """out_sb = relu(ps * cos + pairswap(ps) * sinA), [128, 512] over 4 heads.

    sinA carries the pair-rotation signs (host-prepped).  The pair swap is
    folded into two half-width strided DVE muls (even out slots read odd ps
    slots and vice versa); cos/sin tables are broadcast across heads with
    0-stride APs.
    """
    tpool = pools["rtmp"]
    ta = tpool.tile([128, M], F32, tag="ta")
    tb = tpool.tile([128, M], F32, tag="tb")
    # views: p (h n two) with two=pair slot
    pv = ps.rearrange("p (h n two) -> p h n two", h=HPC, two=2)
    tav = ta.rearrange("p (h n two) -> p h n two", h=HPC, two=2)
    sv = sin_t.rearrange("p (n two) -> p n two", two=2)
    sv_e = sv[:, :, 0].unsqueeze(1).broadcast_to([128, HPC, D // 2])
    sv_o = sv[:, :, 1].unsqueeze(1).broadcast_to([128, HPC, D // 2])
    # ta[even] = ps[odd] * sinA[even];  ta[odd] = ps[even] * sinA[odd]
    nc.vector.tensor_mul(tav[:, :, :, 0], pv[:, :, :, 1], sv_e)
    nc.vector.tensor_mul(tav[:, :, :, 1], pv[:, :, :, 0], sv_o)
    cos_b = cos_t.unsqueeze(1).broadcast_to([128, HPC, D])
    nc.vector.tensor_mul(tb.rearrange("p (h d) -> p h d", d=D),
                         ps.rearrange("p (h d) -> p h d", d=D), cos_b)
    nc.vector.tensor_add(ta, tb, ta)
    nc.scalar.activation(out=out_sb, in_=ta,
                         func=mybir.ActivationFunctionType.Relu)


def build_nc():
    nc = bacc.Bacc(None, target_bir_lowering=False)
    xT = nc.declare_dram_parameter("xT", [C, S], F32R, isOutput=False)
    wqT = nc.declare_dram_parameter("wqT", [C, M], F32R, isOutput=False)
    wkT = nc.declare_dram_parameter("wkT", [C, M], F32R, isOutput=False)
    wvT = nc.declare_dram_parameter("wvT", [C, M], F32R, isOutput=False)
    woT = nc.declare_dram_parameter("woT", [M, C], F32R, isOutput=False)
    cosd = nc.declare_dram_parameter("cos", [S, D], F32, isOutput=False)
    sind = nc.declare_dram_parameter("sinA", [S, D], F32, isOutput=False)
    outT = nc.declare_dram_parameter("outT", [C, S], F32, isOutput=True)

    with tile.TileContext(nc) as tc, ExitStack() as top:
        persist = top.enter_context(tc.tile_pool(name="persist", bufs=1))

        # qrT stash lives in SBUF for the whole kernel: [d, h, s]
        qrT_sb = persist.tile([128, HPC, S], F32R, tag="qrT")
        ones4 = persist.tile([128, HPC], F32, tag="ones4")
        nc.vector.memset(ones4, 1.0)
        ident = persist.tile([128, 128], F32, tag="ident")
        make_identity(nc, ident)
        identr = persist.tile([128, 128], F32R, tag="identr")
        nc.vector.tensor_copy(out=identr, in_=ident)
        vk_sb = persist.tile([128, HPC, D + 1], F32R, tag="vksb")

        # ---------------- phase 1 ----------------
        with ExitStack() as p1:
            wpool = p1.enter_context(tc.tile_pool(name="p1w", bufs=1))
            wq_sb = wpool.tile([128, CT, M], F32R, tag="wq")
            wk_sb = wpool.tile([128, CT, M], F32R, tag="wk")
            wv_sb = wpool.tile([128, CT, M], F32R, tag="wv")
            # The cost model serializes all DMA on one exclusive channel, so
            # warmup is bounded by the ~36us of weight streaming.  Issue x0
            # first (SP), then wq quartered + wk + wv on the ACT queue, and
            # emit tiles 0..2 in an order (q,q,T,q,T,k,k,k,v,vk,...) that
            # keeps the in-order PE busy inside each weight-wait window.
            xpool = p1.enter_context(tc.tile_pool(name="p1x", bufs=3))
            cpool = p1.enter_context(tc.tile_pool(name="p1c", bufs=3))
            qkpool = p1.enter_context(tc.tile_pool(name="p1qk", bufs=2))
            vpool = p1.enter_context(tc.tile_pool(name="p1v", bufs=1))
            rtmp = p1.enter_context(tc.tile_pool(name="p1t", bufs=1))
            qkvps = p1.enter_context(
                tc.tile_pool(name="p1ps", bufs=2, space="PSUM"))
            vkps_pool = p1.enter_context(
                tc.tile_pool(name="p1vk", bufs=4, space="PSUM"))
            tppool = p1.enter_context(
                tc.tile_pool(name="p1tp", bufs=2, space="PSUM"))
            pools = {"rtmp": rtmp}

            vkps = [vkps_pool.tile([128, 2 * D], F32, tag="vkp",
                                   name=f"vkps{h}") for h in range(HPC)]

            def dma_tile(ti, queue=None, split=1):
                ssl = slice(ti * 128, (ti + 1) * 128)
                xt = xpool.tile([128, CT, 128], F32R, tag="xt")
                xin = xT[:, ssl].rearrange("(po pi) f -> pi po f", pi=128)
                for cq in range(split):
                    cs = slice(cq * (CT // split), (cq + 1) * (CT // split))
                    (queue or nc.sync).dma_start(out=xt[:, cs, :],
                                                 in_=xin[:, cs, :])
                cos_t = cpool.tile([128, D], F32, tag="cos")
                sin_t = cpool.tile([128, D], F32, tag="sin")
                nc.gpsimd.dma_start(out=cos_t, in_=cosd[ssl, :])
                nc.gpsimd.dma_start(out=sin_t, in_=sind[ssl, :])
                return xt, cos_t, sin_t

            def proj(xt, wsb):
                ps = qkvps.tile([128, M], F32, tag="pqkv")
                for ci in range(CT):
                    nc.tensor.matmul(ps, lhsT=xt[:, ci, :],
                                     rhs=wsb[:, ci, :],
                                     start=(ci == 0), stop=(ci == CT - 1))
                return ps

            def rope(ps, cos_t, sin_t, dt_, tag):
                out = qkpool.tile([128, M], dt_, tag=tag)
                _rope_relu(nc, pools, ps, cos_t, sin_t, out)
                return out

            def vpad(psv):
                # v padded per head to 256 cols: [v_h | 1 | zeros].  The ones
                # column shares the accumulation group (PSUM start clears the
                # whole bank); the zero tail pads the matmul free dim to 256
                # so the f32r vk contraction runs at 1 cycle/row instead of 4.
                v_sb = vpool.tile([128, HPC * 2 * D], F32R, tag="v")
                v_vw = v_sb.rearrange("p (h n) -> p h n", n=2 * D)
                nc.scalar.copy(out=v_vw[:, :, D:D + 1],
                               in_=ones4.unsqueeze(-1))
                nc.scalar.activation(
                    out=v_vw[:, :, D + 1:2 * D],
                    in_=ones4.unsqueeze(-1).broadcast_to([128, HPC, D - 1]),
                    func=mybir.ActivationFunctionType.Copy, scale=0.0)
                nc.scalar.copy(out=v_vw[:, :, 0:D],
                               in_=psv.rearrange("p (h d) -> p h d", d=D))
                return v_vw

            def vk_acc(ti, kr, v_vw):
                first, last = ti == 0, ti == ST - 1
                for h in range(HPC):
                    nc.tensor.matmul(vkps[h], lhsT=kr[:, h * D:(h + 1) * D],
                                     rhs=v_vw[:, h, :], start=first,
                                     stop=last)

            def stash_q(ti, qr):
                ssl = slice(ti * 128, (ti + 1) * 128)
                for h in range(HPC):
                    tp = tppool.tile([128, 128], F32R, tag="tp")
                    nc.tensor.transpose(tp, qr[:, h * D:(h + 1) * D], identr)
                    nc.scalar.copy(out=qrT_sb[:, h, ssl], in_=tp)

            # warmup: x0 ahead of the weight stream, then tiles 0..2 with
            # q-projs and transposes filling the wk/wv wait windows
            x0, c0, s0 = dma_tile(0, split=4)
            for cq in range(16):
                cs = slice(cq * (CT // 16), (cq + 1) * (CT // 16))
                nc.scalar.dma_start(
                    out=wq_sb[:, cs, :],
                    in_=wqT[:, :].rearrange("(po pi) m -> pi po m",
                                            pi=128)[:, cs, :])
            x1, c1, s1 = dma_tile(1, queue=nc.scalar)
            x2, c2, s2 = dma_tile(2, queue=nc.scalar)
            for cq in range(4):
                cs = slice(cq * (CT // 4), (cq + 1) * (CT // 4))
                nc.scalar.dma_start(
                    out=wk_sb[:, cs, :],
                    in_=wkT[:, :].rearrange("(po pi) m -> pi po m",
                                            pi=128)[:, cs, :])
            for cq in range(2):
                cs = slice(cq * (CT // 2), (cq + 1) * (CT // 2))
                nc.scalar.dma_start(
                    out=wv_sb[:, cs, :],
                    in_=wvT[:, :].rearrange("(po pi) m -> pi po m",
                                            pi=128)[:, cs, :])
            q0 = rope(proj(x0, wq_sb), c0, s0, F32R, "qr")
            q1 = rope(proj(x1, wq_sb), c1, s1, F32R, "qr")
            stash_q(0, q0)
            q2 = rope(proj(x2, wq_sb), c2, s2, F32R, "qr")
            stash_q(1, q1)
            k0 = rope(proj(x0, wk_sb), c0, s0, F32R, "kr")
            k1 = rope(proj(x1, wk_sb), c1, s1, F32R, "kr")
            k2 = rope(proj(x2, wk_sb), c2, s2, F32R, "kr")
            vv0 = vpad(proj(x0, wv_sb))
            vk_acc(0, k0, vv0)
            vv1 = vpad(proj(x1, wv_sb))
            vk_acc(1, k1, vv1)
            vv2 = vpad(proj(x2, wv_sb))
            vk_acc(2, k2, vv2)
            stash_q(2, q2)

            for ti in range(3, ST):
                xt, cos_t, sin_t = dma_tile(ti)
                qr = rope(proj(xt, wq_sb), cos_t, sin_t, F32R, "qr")
                kr = rope(proj(xt, wk_sb), cos_t, sin_t, F32R, "kr")
                v_vw = vpad(proj(xt, wv_sb))
                vk_acc(ti, kr, v_vw)
                if ti == ST - 1:
                    # drain the accumulators before the last stash so the
                    # phase-2 den chains start as early as possible
                    for h in range(HPC):
                        nc.vector.tensor_copy(out=vk_sb[:, h, :],
                                              in_=vkps[h][:, 0:D + 1])
                stash_q(ti, qr)

        # ---------------- phase 2 ----------------
        with ExitStack() as p2:
            wopool = p2.enter_context(tc.tile_pool(name="p2w", bufs=1))
            wo_sb = wopool.tile([128, HPC, C], F32R, tag="wo")
            # split the load so the first out-proj j-tiles don't wait on the
            # full 4MB transfer
            for jq in range(4):
                jsl = slice(jq * (C // 4), (jq + 1) * (C // 4))
                nc.scalar.dma_start(
                    out=wo_sb[:, :, jsl],
                    in_=woT[:, jsl].rearrange("(po pi) j -> pi po j", pi=128))

            # wT = (Wo_h @ vk_h).T precompute: since the per-s normalization
            # scales COLUMNS of hs, it commutes past Wo — so fold vk into the
            # out-projection weights once (wT[d, c] = sum_n vk[n,d] Wo[c,n])
            # and apply the normalization to qT instead.  This deletes the
            # per-chunk hs matmuls and gives the PE the wT build to chew on
            # while the wo stream and the first den chains are in flight.
            vkT_sb = wopool.tile([128, HPC, D], F32R, tag="vkT")

            hsnpool = p2.enter_context(tc.tile_pool(name="p2n", bufs=16))
            rbpool = p2.enter_context(tc.tile_pool(name="p2b", bufs=7))
            tpps2 = p2.enter_context(
                tc.tile_pool(name="p2tp", bufs=2, space="PSUM"))
            opps = p2.enter_context(
                tc.tile_pool(name="p2o", bufs=6, space="PSUM"))

            # vk.T per head (PE transposes; vk_sb is [d, n], wT needs [n, d])
            for h in range(HPC):
                tp = tpps2.tile([128, 128], F32R, tag="vkT")
                nc.tensor.transpose(tp, vk_sb[:, h, 0:D], identr)
                nc.scalar.copy(out=vkT_sb[:, h, :], in_=tp)

            def build_wT(jq):
                # in-place: the matmul reads the wo quarter, then the copy
                # overwrites it with wT (wo is dead afterwards) — saves a
                # whole 32KB wT tile
                jsl = slice(jq * (C // 4), (jq + 1) * (C // 4))
                for h in range(HPC):
                    po = opps.tile([128, 512], F32, tag="po")
                    nc.tensor.matmul(po, lhsT=vkT_sb[:, h, :],
                                     rhs=wo_sb[:, h, jsl],
                                     start=True, stop=True)
                    nc.scalar.copy(out=wo_sb[:, h, jsl], in_=po)

            def do_qhat(sc):
                # den via DVE mul + gpsimd partition_all_reduce (no PE);
                # normalization applied to qT directly
                csl = slice(sc * 512, (sc + 1) * 512)
                qh = []
                for h in range(HPC):
                    qt = qrT_sb[:, h, csl]
                    tmp = rbpool.tile([128, 512], F32, tag="dtmp")
                    nc.vector.tensor_scalar(
                        out=tmp, in0=qt.bitcast(F32),
                        scalar1=vk_sb[:, h, D:D + 1].bitcast(F32),
                        scalar2=EPS / 128.0,
                        op0=mybir.AluOpType.mult, op1=mybir.AluOpType.add)
                    dnb = rbpool.tile([128, 512], F32, tag="dnb")
                    nc.gpsimd.partition_all_reduce(
                        dnb, tmp, channels=128,
                        reduce_op=bass_isa.ReduceOp.add)
                    rb = rbpool.tile([128, 512], F32, tag="rb")
                    nc.vector.reciprocal(out=rb, in_=dnb)
                    qn = hsnpool.tile([128, 512], F32R, tag="qh")
                    nc.vector.tensor_mul(qn, qt.bitcast(F32), rb)
                    qh.append(qn)
                return qh

            def do_outproj_part(sc, jq, qh):
                csl = slice(sc * 512, (sc + 1) * 512)
                for j in range(jq * 4, jq * 4 + 4):
                    po = opps.tile([128, 512], F32, tag="po")
                    for h in range(HPC):
                        nc.tensor.matmul(
                            po, lhsT=wo_sb[:, h, j * 128:(j + 1) * 128],
                            rhs=qh[h], start=(h == 0), stop=(h == HPC - 1))
                    osb = rbpool.tile([128, 512], F32, tag="osb")
                    nc.scalar.copy(out=osb, in_=po)
                    nc.sync.dma_start(
                        out=outT[j * 128:(j + 1) * 128, csl], in_=osb)

            # fill: interleave the wT build with chunk-0's out-projection so
            # each wo quarter is consumed as it arrives
            AHEAD = 3
            qh_q = [do_qhat(sc) for sc in range(AHEAD)]
            for jq in range(4):
                build_wT(jq)
                do_outproj_part(0, jq, qh_q[0])
            for sc in range(1, NCH):
                if sc + AHEAD - 1 < NCH:
                    qh_q.append(do_qhat(sc + AHEAD - 1))
                for jq in range(4):
                    do_outproj_part(sc, jq, qh_q[sc])

    nc.finalize()
    return nc


def _get_nc():
    if "nc" not in _CACHE:
        _CACHE["nc"] = build_nc()
    return _CACHE["nc"]


def _prep_in_maps(x, cos, sin, Wq, Wk, Wv, Wo):
    f = np.float32
    sinA = np.array(sin, dtype=f)
    sinA[:, 0::2] *= -1.0
    cosf = np.ascontiguousarray(np.asarray(cos, f))
    in_maps = []
    for core in range(NCORES):
        b = core // TPG
        r = core % TPG
        rows = slice(r * M, (r + 1) * M)
        in_maps.append({
            "xT": np.ascontiguousarray(np.asarray(x[b], f).T),
            "wqT": np.ascontiguousarray(np.asarray(Wq, f)[rows, :].T),
            "wkT": np.ascontiguousarray(np.asarray(Wk, f)[rows, :].T),
            "wvT": np.ascontiguousarray(np.asarray(Wv, f)[rows, :].T),
            "woT": np.ascontiguousarray(np.asarray(Wo, f)[:, rows].T),
            "cos": cosf,
            "sinA": sinA,
        })
    return in_maps


def kernel(x, cos, sin, Wq, bq, Wk, bk, Wv, bv, Wo, bo):
    from concourse.bass_utils import run_bass_kernel_spmd

    nc = _get_nc()
    in_maps = _prep_in_maps(x, cos, sin, Wq, Wk, Wv, Wo)
    trace = os.environ.get("KERNEL_TRACE", "0") == "1"
    kw = {}
    if trace:
        kw["trace"] = True
        tdir = os.environ.get("KERNEL_TRACE_DIR")
        if tdir:
            os.makedirs(tdir, exist_ok=True)
            kw["tmpdir"] = tdir
    t0 = time.time()
    res = run_bass_kernel_spmd(nc, in_maps, list(range(NCORES)), **kw)
    t1 = time.time()
    LAST_STATS["wall_s"] = t1 - t0
    LAST_STATS["exec_time_ns"] = res.exec_time_ns
    LAST_STATS["trace"] = res.instructions_and_trace

    out = np.empty((B, S, C), np.float32)
    bo32 = np.asarray(bo, np.float32)
    for b in range(B):
        acc = res.results[b * TPG]["outT"].astype(np.float64)
        for r in range(1, TPG):
            acc += res.results[b * TPG + r]["outT"]
        out[b] = acc.T.astype(np.float32) + bo32
    return out
